# revision 1
# baseline (speedup 1.0000x reference)
"""Trainium2 Bass kernel for nn_LiveNet (2-layer MLP: relu(x@W1+b1)@W2+b2).

Sharding: pure data-parallel over batch across 8 NeuronCores (no
collectives).  Each core computes y_i = relu(x_i @ W1 + b1) @ W2 + b2 for
its 512-row batch shard.

Per-core dataflow (all matmuls run on the PE array; contraction dim is
always the SBUF partition dim):
  GEMM1: hT[hid, batch] tiles: lhsT = W1[k-tile, m-chunk] (stationary),
         rhs = xT[k-tile, :] (moving, N=512).  PSUM accumulates over the
         8 k-tiles, then ScalarE applies bias+ReLU while evicting to SBUF.
  GEMM2: y[batch, out]: lhsT = hT[k-tile, batch-chunk], rhs = W2[k-tile,
         n-chunk].  8 PSUM banks accumulate (4 batch x 2 out chunks) over
         32 k-tiles while W2 streams from HBM; VectorE adds b2 on evict.

Host side does layout-only transforms (shard/transpose/retile) so every
DMA is >=2KB-contiguous per partition.  Matmul operands are bitcast to
float32r: full fp32 operand bytes, runs at 1 cycle/row (bf16 speed) for
moving dim >= 256.
"""

import os
import sys

import numpy as np

for _p in ("/opt/trn_rl_repo", "/root/.axon_site/_ro/trn_rl_repo"):
    if os.path.isdir(_p) and _p not in sys.path:
        sys.path.append(_p)

import concourse.bacc as bacc
import concourse.bass as bass
import concourse.tile as tile
from concourse import mybir
from concourse.bass_utils import run_bass_kernel_spmd

N_CORES = 8
B, N_IN, N_HID, N_OUT = 4096, 1024, 4096, 1024
BSH = B // N_CORES          # 512 batch rows per core
P = 128                     # SBUF partitions
KT1 = N_IN // P             # 8  k-tiles in GEMM1
MT1 = N_HID // P            # 32 m-tiles (hid) in GEMM1
KT2 = N_HID // P            # 32 k-tiles in GEMM2
MT2 = BSH // P              # 4  batch m-tiles in GEMM2
NCH = 512                   # moving free dim per matmul
NT2 = N_OUT // NCH          # 2  out chunks in GEMM2

F32 = mybir.dt.float32
F32R = mybir.dt.float32r
RELU = mybir.ActivationFunctionType.Relu


def build_nc(reps=1):
    nc = bacc.Bacc("TRN2", target_bir_lowering=False, debug=False,
                   num_devices=N_CORES)

    xt = nc.declare_dram_parameter("xt", [N_IN, BSH], F32R, isOutput=False)
    w1r = nc.declare_dram_parameter("w1r", [MT1, P, N_IN], F32R, isOutput=False)
    w2 = nc.declare_dram_parameter("w2", [N_HID, N_OUT], F32R, isOutput=False)
    b1t = nc.declare_dram_parameter("b1t", [P, MT1], F32, isOutput=False)
    b2r = nc.declare_dram_parameter("b2r", [P, N_OUT], F32, isOutput=False)
    y = nc.declare_dram_parameter("y", [BSH, N_OUT], F32, isOutput=True)

    with tile.TileContext(nc) as tc:
        with (
            tc.tile_pool(name="const", bufs=1) as const,
            tc.tile_pool(name="xt", bufs=1) as xt_pool,
            tc.tile_pool(name="ht", bufs=1) as ht_pool,
            tc.tile_pool(name="w1", bufs=4) as w1_pool,
            tc.tile_pool(name="w2", bufs=4) as w2_pool,
            tc.tile_pool(name="yout", bufs=4) as y_pool,
            tc.tile_pool(name="ps", bufs=8, space=bass.MemorySpace.PSUM) as ps_pool,
        ):
            b1_sb = const.tile([P, MT1], F32)
            nc.sync.dma_start(out=b1_sb[:], in_=b1t[:])
            # Prime ACT with the bias-load DMA wait so later evict
            # instructions (which already wait on the PE sem) don't exceed
            # the per-instruction sync-wait budget in walrus codegen.
            prime1 = const.tile([P, 1], F32)
            nc.scalar.activation(
                prime1[:], b1_sb[:, 0:1], mybir.ActivationFunctionType.Copy
            )
            b2_sb = const.tile([P, N_OUT], F32)

            for rep in range(reps):
                # Prefetch the first W1 chunks ahead of the bulk xT load so
                # the first matmul's operands land as early as possible.
                w1_head = []
                for m in range(3):
                    w1_sb = w1_pool.tile([P, N_IN], F32R, tag="w1",
                                         name="w1_sb")
                    nc.sync.dma_start(out=w1_sb[:], in_=w1r[m])
                    w1_head.append(w1_sb)

                # xT in SBUF as one tile per k so the first matmul only
                # waits on its own 256KB slice (startup-latency fix).
                xt_sb = [
                    xt_pool.tile([P, BSH], F32R, tag=f"xtk_{k}",
                                 name=f"xtk_{k}")
                    for k in range(KT1)
                ]
                for k in range(KT1):
                    nc.sync.dma_start(
                        out=xt_sb[k][:], in_=xt[k * P:(k + 1) * P, :]
                    )

                # hT resident: [128, 32, 512]; tile j = hid rows j*128..+127.
                ht_sb = ht_pool.tile([P, MT1, BSH], F32R, tag="ht",
                                     name="ht_sb")

                # ---- GEMM1: hT = relu(W1.T-tiled @ xT + b1) ----
                for m in range(MT1):
                    if m < 3:
                        w1_sb = w1_head[m]
                    else:
                        w1_sb = w1_pool.tile([P, N_IN], F32R, tag="w1",
                                             name="w1_sb")
                        nc.sync.dma_start(out=w1_sb[:], in_=w1r[m])
                    ps = ps_pool.tile([P, BSH], F32, tag="ps", name="ps")
                    for k in range(KT1):
                        nc.tensor.matmul(
                            ps[:],
                            w1_sb[:, k * P:(k + 1) * P],
                            xt_sb[k][:],
                            start=(k == 0),
                            stop=(k == KT1 - 1),
                        )
                    nc.scalar.activation(
                        ht_sb[:, m, :], ps[:], RELU, bias=b1_sb[:, m:m + 1],
                    )

                if rep == 0:
                    # b2 is only needed by the final evicts — load it after
                    # the startup DMA burst so it doesn't steal bandwidth
                    # from the first matmul's operands.
                    nc.sync.dma_start(out=b2_sb[:], in_=b2r[:])
                    prime2 = const.tile([P, 1], F32)
                    nc.vector.tensor_copy(prime2[:], b2_sb[:, 0:1])

                # ---- GEMM2: y = hT.T @ W2 + b2 ----
                ps2 = [
                    ps_pool.tile([P, NCH], F32, tag="ps", name=f"ps2_{i}")
                    for i in range(MT2 * NT2)
                ]
                for k in range(KT2):
                    w2_sb = w2_pool.tile([P, N_OUT], F32R, tag="w2",
                                         name="w2_sb")
                    nc.sync.dma_start(out=w2_sb[:], in_=w2[k * P:(k + 1) * P, :])
                    for m in range(MT2):
                        for n in range(NT2):
                            nc.tensor.matmul(
                                ps2[m * NT2 + n][:],
                                ht_sb[:, k, m * P:(m + 1) * P],
                                w2_sb[:, n * NCH:(n + 1) * NCH],
                                start=(k == 0),
                                stop=(k == KT2 - 1),
                            )
                for m in range(MT2):
                    for n in range(NT2):
                        y_sb = y_pool.tile([P, NCH], F32, tag="y",
                                           name="y_sb")
                        nc.vector.tensor_add(
                            y_sb[:], ps2[m * NT2 + n][:],
                            b2_sb[:, n * NCH:(n + 1) * NCH],
                        )
                        nc.sync.dma_start(
                            out=y[m * P:(m + 1) * P, n * NCH:(n + 1) * NCH],
                            in_=y_sb[:],
                        )
    nc.compile()
    return nc


def _prep_shared(W1, b1, W2, b2):
    W1 = np.ascontiguousarray(W1, dtype=np.float32)
    # w1r[m, p, k*128+c] = W1[k*128+p, m*128+c]
    w1r = np.ascontiguousarray(
        W1.reshape(KT1, P, MT1, P).transpose(2, 1, 0, 3)
    ).reshape(MT1, P, N_IN)
    b1t = np.ascontiguousarray(
        np.asarray(b1, dtype=np.float32).reshape(MT1, P).T
    )
    b2r = np.ascontiguousarray(
        np.broadcast_to(np.asarray(b2, dtype=np.float32), (P, N_OUT))
    )
    w2 = np.ascontiguousarray(W2, dtype=np.float32)
    return w1r, b1t, w2, b2r


def kernel(x, W1, b1, W2, b2):
    x = np.ascontiguousarray(x, dtype=np.float32)
    w1r, b1t, w2, b2r = _prep_shared(W1, b1, W2, b2)

    in_maps = []
    for i in range(N_CORES):
        xt_i = np.ascontiguousarray(x[i * BSH:(i + 1) * BSH, :].T)
        in_maps.append(
            {"xt": xt_i, "w1r": w1r, "w2": w2, "b1t": b1t, "b2r": b2r}
        )

    nc = build_nc()
    res = run_bass_kernel_spmd(nc, in_maps, list(range(N_CORES)))
    y = np.concatenate(
        [np.asarray(res.results[i]["y"]) for i in range(N_CORES)], axis=0
    )
    return y.astype(np.float32)


if __name__ == "__main__":
    rng = np.random.default_rng(0)
    x = rng.standard_normal((B, N_IN), dtype=np.float32)
    W1 = rng.standard_normal((N_IN, N_HID), dtype=np.float32) / 32
    b1 = rng.standard_normal((N_HID,), dtype=np.float32) / 32
    W2 = rng.standard_normal((N_HID, N_OUT), dtype=np.float32) / 64
    b2 = rng.standard_normal((N_OUT,), dtype=np.float32) / 64
    y = kernel(x, W1, b1, W2, b2)
    h = np.maximum(x @ W1 + b1, 0)
    y_ref = h @ W2 + b2
    err = np.linalg.norm(y - y_ref) / np.linalg.norm(y_ref)
    print("rel_l2:", err)



# revision 9
# speedup vs baseline: 1.1140x; 1.1140x over previous
"""Trainium2 Bass kernel for nn_LiveNet (2-layer MLP: relu(x@W1+b1)@W2+b2).

Sharding: pure data-parallel over batch across 8 NeuronCores (no
collectives).  Each core computes y_i = relu(x_i @ W1 + b1) @ W2 + b2 for
its 512-row batch shard.

Design (v3) -- all matmul operands are fp16 (PE runs fp16 at 1 row/cycle,
same rate as fp32r, but DMA bytes halve; rel err ~3e-4 vs the fp32
reference):

  GEMM1 (k-outer, m-group inner): hidden cols are processed in 8 groups of
  512 (4 PSUM banks per group, rotating through the 8 banks).  For group
  g, step k accumulates 4 matmuls lhsT=W1[k-slice, g-cols], rhs=xT[k-tile].
  A step needs only 0.13MB of W1, so the DMA stream (shared ~360GB/s pipe)
  feeds the PE without stalls from the first tile on.  ACT evicts each
  bank with fused bias+ReLU to fp16 hT (one SBUF tile per hid k-tile so
  GEMM2 step k depends only on evict k).

  GEMM2 (k-inner per output tile): W2 is fully resident in SBUF (fp16,
  streamed during GEMM1).  Each of the 9 output tiles (4 batch x 2 col
  chunks, last chunk split 384+128 to shorten the drain) accumulates its
  full 32-step k sweep in one PSUM bank, then DVE evicts with +b2 and the
  y tile DMAs out (ACT queue) immediately -- the drain tail is ~3us.

  DMA discipline: the issuing engine's sequencer is held for roughly the
  transfer duration, so all input DMAs go on one SP stream ordered exactly
  by PE need, with per-partition-contiguous >=2KB elements (w1/w2 tiles
  are laid out [partition][k*cols] on the host).  y DMAs ride the ACT
  queue.  W1 group 1 is split into two half DMAs so group 1's first
  matmul isn't gated on the full 0.75MB tile.
"""

import os
import sys

import numpy as np

for _p in ("/opt/trn_rl_repo", "/root/.axon_site/_ro/trn_rl_repo"):
    if os.path.isdir(_p) and _p not in sys.path:
        sys.path.append(_p)

import concourse.bacc as bacc
import concourse.bass as bass
import concourse.tile as tile
from concourse import mybir
from concourse.bass_utils import run_bass_kernel_spmd

N_CORES = 8
B, N_IN, N_HID, N_OUT = 4096, 1024, 4096, 1024
BSH = B // N_CORES          # 512 batch rows per core
P = 128                     # SBUF partitions
KT1 = N_IN // P             # 8  k-tiles in GEMM1
NG1 = 8                     # hid groups in GEMM1 (512 cols each)
MPG = 4                     # m-tiles (PSUM banks) per group
KT2 = N_HID // P            # 32 k-tiles in GEMM2
MT2 = BSH // P              # 4  batch tiles in GEMM2
NCH = 512                   # out-col chunk in GEMM2
MT1 = N_HID // P            # 32 hT k-tiles

N_WARMUP = 0                # PE warmup matmuls (p-state ramp is wall-clock)

F32 = mybir.dt.float32
F16 = mybir.dt.float16
RELU = mybir.ActivationFunctionType.Relu


def build_nc(reps=1):
    nc = bacc.Bacc("TRN2", target_bir_lowering=False, debug=False,
                   num_devices=N_CORES)

    # c0[k] = [W1 k-slice of group0 (4 m-slices, 512 cols) | xT k-tile]
    c0 = nc.declare_dram_parameter("c0", [KT1, P, 2 * NCH], F16, isOutput=False)
    # w1gc[g-1] for groups 1..7: [p, k*512+j] (partition-contiguous)
    w1gc = nc.declare_dram_parameter("w1gc", [NG1 - 1, P, KT1 * NCH], F16,
                                     isOutput=False)
    # w2rc[n, q] = [p, kk*512+c] for k-tiles q*8..q*8+7 of W2 cols n*512..
    w2rc = nc.declare_dram_parameter("w2rc", [2, 4, P, 8 * NCH], F16,
                                     isOutput=False)
    b1t = nc.declare_dram_parameter("b1t", [P, MT1], F32, isOutput=False)
    b2r = nc.declare_dram_parameter("b2r", [P, N_OUT], F32, isOutput=False)
    y = nc.declare_dram_parameter("y", [BSH, N_OUT], F32, isOutput=True)

    with tile.TileContext(nc) as tc:
        with (
            tc.tile_pool(name="const", bufs=1) as const,
            tc.tile_pool(name="c0", bufs=1) as c0_pool,
            tc.tile_pool(name="w1", bufs=1) as w1_pool,
            tc.tile_pool(name="w2", bufs=1) as w2_pool,
            tc.tile_pool(name="ht", bufs=1) as ht_pool,
            tc.tile_pool(name="yout", bufs=3) as y_pool,
            tc.tile_pool(name="ps", bufs=8, space=bass.MemorySpace.PSUM) as ps_pool,
        ):
            if N_WARMUP:
                junk = const.tile([P, P], F16, name="junk")
                nc.vector.memset(junk[:], 0.0)
                ps_w = ps_pool.tile([P, NCH], F32, tag="ps", name="ps_warm")
                for w in range(N_WARMUP):
                    nc.tensor.matmul(
                        ps_w[:, 0:P], junk[:], junk[:],
                        start=(w == 0), stop=(w == N_WARMUP - 1),
                    )

            for rep in range(reps):
                c0_sb = [
                    c0_pool.tile([P, 2 * NCH], F16, tag=f"c0_{k}",
                                 name=f"c0_{k}")
                    for k in range(KT1)
                ]
                if rep == 0:
                    b1_sb = const.tile([P, MT1], F32, name="b1_sb")
                # group 1 split into two half tiles; groups 2..7 whole
                w1h_sb = [
                    w1_pool.tile([P, 4 * NCH], F16, tag=f"w1h_{h}",
                                 name=f"w1h_{h}")
                    for h in range(2)
                ]
                w1_sb = [None, None] + [
                    w1_pool.tile([P, KT1 * NCH], F16, tag=f"w1g_{g}",
                                 name=f"w1g_{g}")
                    for g in range(2, NG1)
                ]
                w2_sb = [
                    w2_pool.tile([P, KT2 * NCH], F16, tag=f"w2n_{n}",
                                 name=f"w2n_{n}")
                    for n in range(2)
                ]
                if rep == 0:
                    b2_sb = const.tile([P, N_OUT], F32, name="b2_sb")

                # -- SP input stream, in exact PE-need order --
                for k in range(KT1):
                    nc.sync.dma_start(out=c0_sb[k][:], in_=c0[k])
                if rep == 0:
                    nc.sync.dma_start(out=b1_sb[:], in_=b1t[:])
                nc.sync.dma_start(out=w1h_sb[0][:], in_=w1gc[0, :, 0:4 * NCH])
                nc.sync.dma_start(out=w1h_sb[1][:],
                                  in_=w1gc[0, :, 4 * NCH:8 * NCH])
                for g in range(2, NG1):
                    nc.sync.dma_start(out=w1_sb[g][:], in_=w1gc[g - 1])
                for q in range(4):
                    nc.sync.dma_start(
                        out=w2_sb[0][:, q * 8 * NCH:(q + 1) * 8 * NCH],
                        in_=w2rc[0, q],
                    )
                if rep == 0:
                    nc.sync.dma_start(out=b2_sb[:], in_=b2r[:])
                for q in range(4):
                    nc.sync.dma_start(
                        out=w2_sb[1][:, q * 8 * NCH:(q + 1) * 8 * NCH],
                        in_=w2rc[1, q],
                    )

                # prime ACT/DVE with the bias-load waits so evicts don't
                # exceed the per-instruction sync-wait budget
                if rep == 0:
                    prime1 = const.tile([P, 1], F32, name="prime1")
                    nc.scalar.activation(
                        prime1[:], b1_sb[:, 0:1],
                        mybir.ActivationFunctionType.Copy,
                    )
                    prime2 = const.tile([P, 1], F32, name="prime2")
                    nc.vector.tensor_copy(prime2[:], b2_sb[:, 0:1])

                # hT resident, one tile per hid k-tile
                ht_sb = [
                    ht_pool.tile([P, BSH], F16, tag=f"ht_{j}", name=f"ht_{j}")
                    for j in range(MT1)
                ]

                # ---- GEMM1: k-outer, 4 banks per hid group ----
                for g in range(NG1):
                    ps = [
                        ps_pool.tile([P, BSH], F32, tag="ps", name=f"ps_{g}_{i}")
                        for i in range(MPG)
                    ]
                    for k in range(KT1):
                        rhs = c0_sb[k][:, NCH:2 * NCH]
                        for i in range(MPG):
                            if g == 0:
                                lhs = c0_sb[k][:, i * P:(i + 1) * P]
                            elif g == 1:
                                half = w1h_sb[k // 4]
                                lhs = half[:, (k % 4) * NCH + i * P:
                                           (k % 4) * NCH + (i + 1) * P]
                            else:
                                lhs = w1_sb[g][:, k * NCH + i * P:
                                               k * NCH + (i + 1) * P]
                            nc.tensor.matmul(
                                ps[i][:],
                                lhs,
                                rhs,
                                start=(k == 0),
                                stop=(k == KT1 - 1),
                            )
                    for i in range(MPG):
                        m = MPG * g + i
                        nc.scalar.activation(
                            ht_sb[m][:], ps[i][:], RELU,
                            bias=b1_sb[:, m:m + 1],
                        )

                # ---- GEMM2: k-inner per output tile, evict+DMA per tile ----
                tiles = []
                for n in range(2):
                    for m in range(MT2):
                        if n == 1 and m == MT2 - 1:
                            tiles.append((m, NCH, 384))
                            tiles.append((m, NCH + 384, 128))
                        else:
                            tiles.append((m, n * NCH, NCH))
                for (m, coff, w) in tiles:
                    n = coff // NCH if coff < 2 * NCH else 1
                    n = 1 if coff >= NCH else 0
                    rel = coff - n * NCH
                    ps2 = ps_pool.tile([P, NCH], F32, tag="ps", name="ps2")
                    for k in range(KT2):
                        nc.tensor.matmul(
                            ps2[:, 0:w],
                            ht_sb[k][:, m * P:(m + 1) * P],
                            w2_sb[n][:, k * NCH + rel:k * NCH + rel + w],
                            start=(k == 0),
                            stop=(k == KT2 - 1),
                        )
                    y_sb = y_pool.tile([P, NCH], F32, tag="y", name="y_sb")
                    nc.vector.tensor_add(
                        y_sb[:, 0:w], ps2[:, 0:w],
                        b2_sb[:, coff:coff + w],
                    )
                    nc.scalar.dma_start(
                        out=y[m * P:(m + 1) * P, coff:coff + w],
                        in_=y_sb[:, 0:w],
                    )
    nc.compile()
    return nc


def _prep_shared(W1, b1, W2, b2):
    W1 = np.ascontiguousarray(W1, dtype=np.float32)
    W2 = np.ascontiguousarray(W2, dtype=np.float32)
    # w1k[g, k, p, j] = W1[k*128+p, g*512+j]
    w1k = W1.reshape(KT1, P, NG1, NCH).transpose(2, 0, 1, 3)
    w1g0 = w1k[0].astype(np.float16)                     # [k, p, 512]
    # w1gc[g-1, p, k*512+j] for g>=1 (partition-contiguous)
    w1gc = np.ascontiguousarray(
        w1k[1:].transpose(0, 2, 1, 3).reshape(NG1 - 1, P, KT1 * NCH),
        dtype=np.float16,
    )
    # w2rc[n, q, p, kk*512+c] = W2[(q*8+kk)*128+p, n*512+c]
    w2rc = np.ascontiguousarray(
        W2.reshape(4, 8, P, 2, NCH).transpose(3, 0, 2, 1, 4)
        .reshape(2, 4, P, 8 * NCH),
        dtype=np.float16,
    )
    b1t = np.ascontiguousarray(
        np.asarray(b1, dtype=np.float32).reshape(MT1, P).T
    )
    b2r = np.ascontiguousarray(
        np.broadcast_to(np.asarray(b2, dtype=np.float32), (P, N_OUT))
    )
    return w1g0, w1gc, w2rc, b1t, b2r


def kernel(x, W1, b1, W2, b2):
    x = np.ascontiguousarray(x, dtype=np.float32)
    w1g0, w1gc, w2rc, b1t, b2r = _prep_shared(W1, b1, W2, b2)

    in_maps = []
    for i in range(N_CORES):
        xs = x[i * BSH:(i + 1) * BSH, :]                 # [512, 1024]
        # xt[k, p, c] = xs[c, k*128+p]
        xt = np.ascontiguousarray(
            xs.T.reshape(KT1, P, BSH), dtype=np.float16
        )
        c0 = np.ascontiguousarray(
            np.concatenate([w1g0, xt], axis=2), dtype=np.float16
        )
        in_maps.append(
            {"c0": c0, "w1gc": w1gc, "w2rc": w2rc, "b1t": b1t, "b2r": b2r}
        )

    nc = build_nc()
    res = run_bass_kernel_spmd(nc, in_maps, list(range(N_CORES)))
    y = np.concatenate(
        [np.asarray(res.results[i]["y"]) for i in range(N_CORES)], axis=0
    )
    return y.astype(np.float32)


if __name__ == "__main__":
    rng = np.random.default_rng(0)
    x = rng.standard_normal((B, N_IN), dtype=np.float32)
    W1 = rng.standard_normal((N_IN, N_HID), dtype=np.float32) / 32
    b1 = rng.standard_normal((N_HID,), dtype=np.float32) / 32
    W2 = rng.standard_normal((N_HID, N_OUT), dtype=np.float32) / 64
    b2 = rng.standard_normal((N_OUT,), dtype=np.float32) / 64
    y = kernel(x, W1, b1, W2, b2)
    h = np.maximum(x @ W1 + b1, 0)
    y_ref = h @ W2 + b2
    err = np.linalg.norm(y - y_ref) / np.linalg.norm(y_ref)
    print("rel_l2:", err)


# revision 28
# speedup vs baseline: 1.1178x; 1.0034x over previous
"""Trainium2 Bass kernel for nn_LiveNet (2-layer MLP: relu(x@W1+b1)@W2+b2).

Sharding: pure data-parallel over batch across 8 NeuronCores (no
collectives).  Each core computes y_i = relu(x_i @ W1 + b1) @ W2 + b2 for
its 512-row batch shard.

Design (v3) -- all matmul operands are fp16 (PE runs fp16 at 1 row/cycle,
same rate as fp32r, but DMA bytes halve; rel err ~3e-4 vs the fp32
reference):

  GEMM1 (k-outer, m-group inner): hidden cols are processed in 8 groups of
  512 (4 PSUM banks per group, rotating through the 8 banks).  For group
  g, step k accumulates 4 matmuls lhsT=W1[k-slice, g-cols], rhs=xT[k-tile].
  A step needs only 0.13MB of W1, so the DMA stream (shared ~360GB/s pipe)
  feeds the PE without stalls from the first tile on.  ACT evicts each
  bank with fused bias+ReLU to fp16 hT (one SBUF tile per hid k-tile so
  GEMM2 step k depends only on evict k).

  GEMM2 (k-inner per output tile): W2 is fully resident in SBUF (fp16,
  streamed during GEMM1).  Each of the 9 output tiles (4 batch x 2 col
  chunks, last chunk split 384+128 to shorten the drain) accumulates its
  full 32-step k sweep in one PSUM bank, then DVE evicts with +b2 and the
  y tile DMAs out (ACT queue) immediately -- the drain tail is ~3us.

  DMA discipline: the issuing engine's sequencer is held for roughly the
  transfer duration, so all input DMAs go on one SP stream ordered exactly
  by PE need, with per-partition-contiguous >=2KB elements (w1/w2 tiles
  are laid out [partition][k*cols] on the host).  y DMAs ride the ACT
  queue.  W1 group 1 is split into two half DMAs so group 1's first
  matmul isn't gated on the full 0.75MB tile.
"""

import os
import sys

import numpy as np

for _p in ("/opt/trn_rl_repo", "/root/.axon_site/_ro/trn_rl_repo"):
    if os.path.isdir(_p) and _p not in sys.path:
        sys.path.append(_p)

import concourse.bacc as bacc
import concourse.bass as bass
import concourse.tile as tile
from concourse import mybir
from concourse.bass_utils import run_bass_kernel_spmd

N_CORES = 8
B, N_IN, N_HID, N_OUT = 4096, 1024, 4096, 1024
BSH = B // N_CORES          # 512 batch rows per core
P = 128                     # SBUF partitions
KT1 = N_IN // P             # 8  k-tiles in GEMM1
NG1 = 8                     # hid groups in GEMM1 (512 cols each)
MPG = 4                     # m-tiles (PSUM banks) per group
KT2 = N_HID // P            # 32 k-tiles in GEMM2
MT2 = BSH // P              # 4  batch tiles in GEMM2
NCH = 512                   # out-col chunk in GEMM2
MT1 = N_HID // P            # 32 hT k-tiles

N_WARMUP = 0                # PE warmup matmuls (p-state ramp is wall-clock)

F32 = mybir.dt.float32
F16 = mybir.dt.float16
RELU = mybir.ActivationFunctionType.Relu


def build_nc(reps=1):
    nc = bacc.Bacc("TRN2", target_bir_lowering=False, debug=False,
                   num_devices=N_CORES)

    # c0[k] = [W1 k-slice of group0 (4 m-slices, 512 cols) | xT k-tile]
    c0 = nc.declare_dram_parameter("c0", [KT1, P, 2 * NCH], F16, isOutput=False)
    # w1gc[g-1] for groups 1..7: [p, k*512+j] partition-major, so every DMA
    # is a shape-matched 2D [P, cols] copy (HW DMA iteration order demands
    # identical in/out AP structure)
    w1gc = nc.declare_dram_parameter("w1gc", [NG1 - 1, P, KT1 * NCH], F16,
                                     isOutput=False)
    # w2rc[n, q] = [p, kk*512+c] for k-tiles q*8..q*8+7 of W2 cols n*512..
    w2rc = nc.declare_dram_parameter("w2rc", [2, 4, P, 8 * NCH], F16,
                                     isOutput=False)
    b1t = nc.declare_dram_parameter("b1t", [P, MT1], F32, isOutput=False)
    b2r = nc.declare_dram_parameter("b2r", [P, N_OUT], F32, isOutput=False)
    y = nc.declare_dram_parameter("y", [BSH, N_OUT], F32, isOutput=True)

    with tile.TileContext(nc) as tc:
        with (
            tc.tile_pool(name="const", bufs=1) as const,
            tc.tile_pool(name="c0", bufs=1) as c0_pool,
            tc.tile_pool(name="w1", bufs=1) as w1_pool,
            tc.tile_pool(name="w2", bufs=1) as w2_pool,
            tc.tile_pool(name="ht", bufs=1) as ht_pool,
            tc.tile_pool(name="yout", bufs=3) as y_pool,
            tc.tile_pool(name="ps", bufs=8, space=bass.MemorySpace.PSUM) as ps_pool,
        ):
            if N_WARMUP:
                junk = const.tile([P, P], F16, name="junk")
                nc.vector.memset(junk[:], 0.0)
                ps_w = ps_pool.tile([P, NCH], F32, tag="ps", name="ps_warm")
                for w in range(N_WARMUP):
                    nc.tensor.matmul(
                        ps_w[:, 0:P], junk[:], junk[:],
                        start=(w == 0), stop=(w == N_WARMUP - 1),
                    )

            for rep in range(reps):
                # c0 k=0 is split into two tiles so the PE can start on the
                # first 0.19MB (w1 slices 2,3 + xT k0) ~0.4us earlier
                c0a_sb = c0_pool.tile([P, 768], F16, tag="c0a", name="c0a")
                c0b_sb = c0_pool.tile([P, 256], F16, tag="c0b", name="c0b")
                c0_sb = [None] + [
                    c0_pool.tile([P, 2 * NCH], F16, tag=f"c0_{k}",
                                 name=f"c0_{k}")
                    for k in range(1, KT1)
                ]
                if rep == 0:
                    b1_sb = const.tile([P, MT1], F32, name="b1_sb")
                # group 1 split k0-1 / k2-3 / k4-7 so each g1 step is gated
                # only on its own slice of the W1 stream
                w1h_sb = [
                    w1_pool.tile([P, 2 * NCH], F16, tag="w1h_0", name="w1h_0"),
                    w1_pool.tile([P, 2 * NCH], F16, tag="w1h_1", name="w1h_1"),
                    w1_pool.tile([P, 4 * NCH], F16, tag="w1h_2", name="w1h_2"),
                ]
                w1_sb = [None, None] + [
                    w1_pool.tile([P, KT1 * NCH], F16, tag=f"w1g_{g}",
                                 name=f"w1g_{g}")
                    for g in range(2, NG1)
                ]
                w2_sb = [
                    w2_pool.tile([P, KT2 * NCH], F16, tag=f"w2n_{n}",
                                 name=f"w2n_{n}")
                    for n in range(2)
                ]
                if rep == 0:
                    b2_sb = const.tile([P, N_OUT], F32, name="b2_sb")

                # -- SP input stream, in exact PE-need order --
                nc.sync.dma_start(out=c0a_sb[:], in_=c0[0, :, 256:2 * NCH])
                nc.sync.dma_start(out=c0b_sb[:], in_=c0[0, :, 0:256])
                for k in range(1, KT1):
                    nc.sync.dma_start(out=c0_sb[k][:], in_=c0[k])
                nc.sync.dma_start(out=w1h_sb[0][:],
                                  in_=w1gc[0, :, 0:2 * NCH])
                nc.sync.dma_start(out=w1h_sb[1][:],
                                  in_=w1gc[0, :, 2 * NCH:4 * NCH])
                nc.sync.dma_start(out=w1h_sb[2][:],
                                  in_=w1gc[0, :, 4 * NCH:8 * NCH])
                if rep == 0:
                    nc.sync.dma_start(out=b1_sb[:], in_=b1t[:])
                for g in range(2, NG1):
                    nc.sync.dma_start(out=w1_sb[g][:], in_=w1gc[g - 1])
                for q in range(4):
                    nc.sync.dma_start(
                        out=w2_sb[0][:, q * 8 * NCH:(q + 1) * 8 * NCH],
                        in_=w2rc[0, q],
                    )
                if rep == 0:
                    nc.sync.dma_start(out=b2_sb[:], in_=b2r[:])
                for q in range(4):
                    nc.sync.dma_start(
                        out=w2_sb[1][:, q * 8 * NCH:(q + 1) * 8 * NCH],
                        in_=w2rc[1, q],
                    )

                # prime ACT/DVE with the bias-load waits so evicts don't
                # exceed the per-instruction sync-wait budget
                if rep == 0:
                    prime1 = const.tile([P, 1], F32, name="prime1")
                    nc.scalar.activation(
                        prime1[:], b1_sb[:, 0:1],
                        mybir.ActivationFunctionType.Copy,
                    )
                    prime2 = const.tile([P, 1], F32, name="prime2")
                    nc.vector.tensor_copy(prime2[:], b2_sb[:, 0:1])

                # hT resident, one tile per hid k-tile
                ht_sb = [
                    ht_pool.tile([P, BSH], F16, tag=f"ht_{j}", name=f"ht_{j}")
                    for j in range(MT1)
                ]

                # ---- GEMM1: k-outer, 4 banks per hid group ----
                for g in range(NG1):
                    ps = [
                        ps_pool.tile([P, BSH], F32, tag="ps", name=f"ps_{g}_{i}")
                        for i in range(MPG)
                    ]
                    for k in range(KT1):
                        if k == 0:
                            rhs = c0a_sb[:, 256:768]
                        else:
                            rhs = c0_sb[k][:, NCH:2 * NCH]
                        order = [2, 3, 0, 1] if (g == 0 and k == 0) else \
                            range(MPG)
                        for i in order:
                            if g == 0 and k == 0:
                                if i >= 2:
                                    lhs = c0a_sb[:, (i - 2) * P:(i - 1) * P]
                                else:
                                    lhs = c0b_sb[:, i * P:(i + 1) * P]
                            elif g == 0:
                                lhs = c0_sb[k][:, i * P:(i + 1) * P]
                            elif g == 1:
                                hsel = min(k // 2, 2)
                                half = w1h_sb[hsel]
                                kr = k - 2 * hsel
                                lhs = half[:, kr * NCH + i * P:
                                           kr * NCH + (i + 1) * P]
                            else:
                                lhs = w1_sb[g][:, k * NCH + i * P:
                                               k * NCH + (i + 1) * P]
                            nc.tensor.matmul(
                                ps[i][:],
                                lhs,
                                rhs,
                                start=(k == 0),
                                stop=(k == KT1 - 1),
                            )
                    for i in range(MPG):
                        m = MPG * g + i
                        nc.scalar.activation(
                            ht_sb[m][:], ps[i][:], RELU,
                            bias=b1_sb[:, m:m + 1],
                        )

                # ---- GEMM2: k-inner per output tile, evict+DMA per tile ----
                tiles = []
                for n in range(2):
                    for m in range(MT2):
                        if n == 1 and m == MT2 - 1:
                            tiles.append((m, NCH, 448))
                            tiles.append((m, NCH + 448, 64))
                        else:
                            tiles.append((m, n * NCH, NCH))
                for ti, (m, coff, w) in enumerate(tiles):
                    n = 1 if coff >= NCH else 0
                    rel = coff - n * NCH
                    ps2 = ps_pool.tile([P, NCH], F32, tag="ps", name="ps2")
                    for k in range(KT2):
                        nc.tensor.matmul(
                            ps2[:, 0:w],
                            ht_sb[k][:, m * P:(m + 1) * P],
                            w2_sb[n][:, k * NCH + rel:k * NCH + rel + w],
                            start=(k == 0),
                            stop=(k == KT2 - 1),
                        )
                    y_sb = y_pool.tile([P, NCH], F32, tag="y", name="y_sb")
                    nc.vector.tensor_add(
                        y_sb[:, 0:w], ps2[:, 0:w],
                        b2_sb[:, coff:coff + w],
                    )
                    # last tile's DMA rides the idle SP queue (650ns DGE
                    # delay vs ACT's 784) to shorten the drain tail
                    eng = nc.sync if ti == len(tiles) - 1 else nc.scalar
                    eng.dma_start(
                        out=y[m * P:(m + 1) * P, coff:coff + w],
                        in_=y_sb[:, 0:w],
                    )
    nc.compile()
    return nc


def _prep_shared(W1, b1, W2, b2):
    W1 = np.ascontiguousarray(W1, dtype=np.float32)
    W2 = np.ascontiguousarray(W2, dtype=np.float32)
    # w1k[g, k, p, j] = W1[k*128+p, g*512+j]
    w1k = W1.reshape(KT1, P, NG1, NCH).transpose(2, 0, 1, 3)
    w1g0 = w1k[0].astype(np.float16)                     # [k, p, 512]
    # w1gc[g-1, p, k*512+j] = W1[k*128+p, g*512+j] (partition-major)
    w1gc = np.ascontiguousarray(
        w1k[1:].transpose(0, 2, 1, 3).reshape(NG1 - 1, P, KT1 * NCH),
        dtype=np.float16,
    )
    # w2rc[n, q, p, kk*512+c] = W2[(q*8+kk)*128+p, n*512+c]
    w2rc = np.ascontiguousarray(
        W2.reshape(4, 8, P, 2, NCH).transpose(3, 0, 2, 1, 4)
        .reshape(2, 4, P, 8 * NCH),
        dtype=np.float16,
    )
    b1t = np.ascontiguousarray(
        np.asarray(b1, dtype=np.float32).reshape(MT1, P).T
    )
    b2r = np.ascontiguousarray(
        np.broadcast_to(np.asarray(b2, dtype=np.float32), (P, N_OUT))
    )
    return w1g0, w1gc, w2rc, b1t, b2r


def kernel(x, W1, b1, W2, b2):
    x = np.ascontiguousarray(x, dtype=np.float32)
    w1g0, w1gc, w2rc, b1t, b2r = _prep_shared(W1, b1, W2, b2)

    in_maps = []
    for i in range(N_CORES):
        xs = x[i * BSH:(i + 1) * BSH, :]                 # [512, 1024]
        # xt[k, p, c] = xs[c, k*128+p]
        xt = np.ascontiguousarray(
            xs.T.reshape(KT1, P, BSH), dtype=np.float16
        )
        c0 = np.ascontiguousarray(
            np.concatenate([w1g0, xt], axis=2), dtype=np.float16
        )
        in_maps.append(
            {"c0": c0, "w1gc": w1gc, "w2rc": w2rc, "b1t": b1t, "b2r": b2r}
        )

    nc = build_nc()
    res = run_bass_kernel_spmd(nc, in_maps, list(range(N_CORES)))
    y = np.concatenate(
        [np.asarray(res.results[i]["y"]) for i in range(N_CORES)], axis=0
    )
    return y.astype(np.float32)


if __name__ == "__main__":
    rng = np.random.default_rng(0)
    x = rng.standard_normal((B, N_IN), dtype=np.float32)
    W1 = rng.standard_normal((N_IN, N_HID), dtype=np.float32) / 32
    b1 = rng.standard_normal((N_HID,), dtype=np.float32) / 32
    W2 = rng.standard_normal((N_HID, N_OUT), dtype=np.float32) / 64
    b2 = rng.standard_normal((N_OUT,), dtype=np.float32) / 64
    y = kernel(x, W1, b1, W2, b2)
    h = np.maximum(x @ W1 + b1, 0)
    y_ref = h @ W2 + b2
    err = np.linalg.norm(y - y_ref) / np.linalg.norm(y_ref)
    print("rel_l2:", err)
